# revision 12
# baseline (speedup 1.0000x reference)
"""DeepSeek sparse-attention decode kernel for Trainium2 (8 NeuronCores).

Problem shapes (hardcoded):
  q       [64, 1, 128, 576] f16   (B, S, H, D+DT)
  kv      [64, 8192, 1, 576] f16  (B, SKV, G, D+DT)
  indices [64, 1, 1, 2048] i32    (B, S, G, K) values in [0, SKV)
  out     [64, 1, 128, 512] f16

Strategy: data-parallel over batch (8 batches per core). Per batch:
  - dma_gather(transpose=True) pulls the 2048 selected kv rows directly in
    [d, k] layout (d on partitions, 5 d-tiles of 128) for the scores matmul.
  - dma_gather(plain) pulls the same rows in [k, d] layout for the AV matmul.
  - scoresT[k, h] = kv_sel @ q^T via 16 k-tiles x 5 d-tiles of 128x128 matmuls.
  - exp on ScalarE produces pT[k, h] straight from PSUM (no max subtraction:
    scaled scores are ~N(0,1), fp32 exp is safe).
  - AV: out[h, d] += pT_i^T @ v_i; Z[h] += pT_i^T @ ones. out = AV / Z.

kv rows are zero-padded to 640 elems host-side so the gather element size
(1280B) and stride meet dma_gather's 256B-divisibility constraints; the pad
contributes 0 to the scores because q is zero-padded too.
"""

import math

import numpy as np

B = 64
S = 1
H = 128
SKV = 8192
D = 512
DT = 64
DK = D + DT  # 576
DPAD = 640
K = 2048
N_CORES = 8
BPC = B // N_CORES  # batches per core
SM_SCALE = 1.0 / math.sqrt(DK)

_PROGRAM_CACHE = {}


def build_program(n_batches=BPC):
    """Build the per-core Bass program (identical on all 8 cores)."""
    import concourse.bacc as bacc
    import concourse.mybir as mybir
    import concourse.tile as tile
    from concourse.library_config import mlp

    f16 = mybir.dt.float16
    f32 = mybir.dt.float32
    i16 = mybir.dt.int16
    Exp = mybir.ActivationFunctionType.Exp

    nc = bacc.Bacc(
        "TRN2",
        target_bir_lowering=False,
        debug=False,
        num_devices=N_CORES,
        num_swdge_queues=4,
        dynamic_dma_scratch_size=1 << 16,
    )
    kvp = nc.dram_tensor("kvp", [n_batches * SKV, DPAD], f16, kind="ExternalInput")
    qt = nc.dram_tensor("qt", [n_batches, 128, DPAD], f16, kind="ExternalInput")
    idx = nc.dram_tensor("idx", [n_batches, 128, K // 16], i16, kind="ExternalInput")
    out = nc.dram_tensor("out", [n_batches, 128, D], f16, kind="ExternalOutput")

    NKT = K // 128  # 16 k-tiles
    NDT = DPAD // 128  # 5 d-tiles
    NG = NKT // 4  # 4 k-groups (one PSUM bank each)

    with tile.TileContext(nc) as tc:
        with (
            tc.tile_pool(name="const", bufs=1) as cpool,
            tc.tile_pool(name="io", bufs=2) as iopool,
            tc.tile_pool(name="gather", bufs=3) as gpool,
            tc.tile_pool(name="pt", bufs=2) as ppool,
            tc.tile_pool(name="sc_ps", bufs=3, space="PSUM") as spsum,
            tc.tile_pool(name="av_ps", bufs=2, space="PSUM") as apsum,
            tc.tile_pool(name="z_ps", bufs=2, space="PSUM") as zpsum,
            tc.tile_pool(name="outp", bufs=2) as opool,
        ):
            nc.gpsimd.load_library(mlp)
            ones = cpool.tile([128, 1], f16)
            nc.vector.memset(ones[:], 1.0)

            for b in range(n_batches):
                idx_t = iopool.tile([128, K // 16], i16, tag="idx")
                nc.sync.dma_start(idx_t[:], idx[b])
                qt_t = iopool.tile([128, DPAD], f16, tag="qt")
                nc.sync.dma_start(qt_t[:], qt[b])

                kv_rows = kvp[b * SKV : (b + 1) * SKV, :]
                # kvT[p, t, i] = kv_row(idx_i)[t*128 + p]
                kvT = gpool.tile([128, NDT, K], f16, tag="kvT")
                nc.gpsimd.dma_gather(
                    kvT[:], kv_rows, idx_t[:], K, K, DPAD, transpose=True,
                    single_packet=False, queue_num=(2 * b) % 4,
                )
                # v[p, j, :] = kv_row(idx_{j*128+p})[0:512]
                v = gpool.tile([128, NKT, D], f16, tag="v")
                nc.gpsimd.dma_gather(
                    v[:], kvp[b * SKV : (b + 1) * SKV, 0:D], idx_t[:], K, K, D,
                    elem_step=DPAD, single_packet=False, queue_num=(2 * b + 1) % 4,
                )

                pT = ppool.tile([128, K], f16, tag="pT")
                for g in range(NG):
                    ps = spsum.tile([128, 512], f32, tag="ps")
                    for j in range(4):
                        i = 4 * g + j
                        for t in range(NDT):
                            nc.tensor.matmul(
                                ps[:, j * 128 : (j + 1) * 128],
                                lhsT=kvT[:, t, i * 128 : (i + 1) * 128],
                                rhs=qt_t[:, t * 128 : (t + 1) * 128],
                                start=(t == 0),
                                stop=(t == NDT - 1),
                            )
                    nc.scalar.activation(
                        pT[:, g * 512 : (g + 1) * 512], ps[:], Exp, scale=SM_SCALE
                    )

                av = apsum.tile([128, D], f32, tag="av")
                zz = zpsum.tile([128, 1], f32, tag="z")
                for i in range(NKT):
                    nc.tensor.matmul(
                        av[:],
                        lhsT=pT[:, i * 128 : (i + 1) * 128],
                        rhs=v[:, i, :],
                        start=(i == 0),
                        stop=(i == NKT - 1),
                    )
                    nc.tensor.matmul(
                        zz[:],
                        lhsT=pT[:, i * 128 : (i + 1) * 128],
                        rhs=ones[:],
                        start=(i == 0),
                        stop=(i == NKT - 1),
                    )

                zr = opool.tile([128, 1], f32, tag="zr")
                nc.vector.reciprocal(zr[:], zz[:])
                ob = opool.tile([128, D], f16, tag="ob")
                nc.vector.tensor_scalar_mul(ob[:], av[:], zr[:])
                nc.sync.dma_start(out[b], ob[:])
    nc.compile()
    return nc


def marshal_core_inputs(q, kv, indices, core):
    """Slice + repack one core's inputs (host-side layout marshaling only)."""
    bs = slice(core * BPC, (core + 1) * BPC)
    kvs = kv[bs, :, 0, :]  # [BPC, SKV, 576]
    kvp = np.zeros((BPC, SKV, DPAD), np.float16)
    kvp[:, :, :DK] = kvs
    kvp = kvp.reshape(BPC * SKV, DPAD)

    qs = q[bs, 0]  # [BPC, 128, 576]
    qtp = np.zeros((BPC, DPAD, 128), np.float16)
    qtp[:, :DK, :] = qs.transpose(0, 2, 1)
    # device layout: qt[b, p, t*128+h] = qT[b, t*128+p, h]
    qt = np.ascontiguousarray(
        qtp.reshape(BPC, DPAD // 128, 128, 128).transpose(0, 2, 1, 3)
    ).reshape(BPC, 128, DPAD)

    idxs = indices[bs, 0, 0, :].astype(np.int16)  # [BPC, K]
    # dma_gather layout: index i lives at partition i%16, column i//16,
    # replicated across the 8 groups of 16 partitions.
    idxw = np.ascontiguousarray(idxs.reshape(BPC, K // 16, 16).transpose(0, 2, 1))
    idxw = np.tile(idxw, (1, 8, 1))  # [BPC, 128, K//16]
    return {"kvp": kvp, "qt": qt, "idx": np.ascontiguousarray(idxw)}


def kernel(q, kv, indices):
    from concourse.bass_utils import run_bass_kernel_spmd

    q = np.asarray(q)
    kv = np.asarray(kv)
    indices = np.asarray(indices)

    if "prog" not in _PROGRAM_CACHE:
        _PROGRAM_CACHE["prog"] = build_program()
    nc = _PROGRAM_CACHE["prog"]

    in_maps = [marshal_core_inputs(q, kv, indices, c) for c in range(N_CORES)]
    res = run_bass_kernel_spmd(nc, in_maps, core_ids=list(range(N_CORES)))
    out = np.stack([r["out"] for r in res.results])  # [8, BPC, 128, 512]
    return out.reshape(B, 128, D)[:, None].astype(np.float16)


# revision 15
# speedup vs baseline: 1.4363x; 1.4363x over previous
"""DeepSeek sparse-attention decode kernel for Trainium2 (8 NeuronCores).

Problem shapes (hardcoded):
  q       [64, 1, 128, 576] f16   (B, S, H, D+DT)
  kv      [64, 8192, 1, 576] f16  (B, SKV, G, D+DT)
  indices [64, 1, 1, 2048] i32    (B, S, G, K) values in [0, SKV)
  out     [64, 1, 128, 512] f16

Strategy: data-parallel over batch (8 batches per core). Per batch:
  - dma_gather(transpose=True) pulls the 2048 selected kv rows directly in
    [d, k] layout (d on partitions, 5 d-tiles of 128) for the scores matmul.
  - dma_gather(plain) pulls the same rows in [k, d] layout for the AV matmul.
  - scoresT[k, h] = kv_sel @ q^T via 16 k-tiles x 5 d-tiles of 128x128 matmuls.
  - exp on ScalarE produces pT[k, h] straight from PSUM (no max subtraction:
    scaled scores are ~N(0,1), fp32 exp is safe).
  - AV: out[h, d] += pT_i^T @ v_i; Z[h] += pT_i^T @ ones. out = AV / Z.

kv rows are zero-padded to 640 elems host-side so the gather element size
(1280B) and stride meet dma_gather's 256B-divisibility constraints; the pad
contributes 0 to the scores because q is zero-padded too.
"""

import math

import numpy as np

B = 64
S = 1
H = 128
SKV = 8192
D = 512
DT = 64
DK = D + DT  # 576
DPAD = 640
K = 2048
N_CORES = 8
BPC = B // N_CORES  # batches per core
SM_SCALE = 1.0 / math.sqrt(DK)

_PROGRAM_CACHE = {}


def build_program(n_batches=BPC):
    """Build the per-core Bass program (identical on all 8 cores)."""
    import concourse.bacc as bacc
    import concourse.mybir as mybir
    import concourse.tile as tile
    from concourse.library_config import mlp

    f16 = mybir.dt.float16
    f32 = mybir.dt.float32
    i16 = mybir.dt.int16
    Exp = mybir.ActivationFunctionType.Exp

    nc = bacc.Bacc(
        "TRN2",
        target_bir_lowering=False,
        debug=False,
        num_devices=N_CORES,
        num_swdge_queues=4,
        dynamic_dma_scratch_size=1 << 16,
    )
    kvp = nc.dram_tensor("kvp", [n_batches * SKV, DPAD], f16, kind="ExternalInput")
    qt = nc.dram_tensor("qt", [n_batches, 128, DPAD], f16, kind="ExternalInput")
    idx = nc.dram_tensor("idx", [n_batches, 128, K // 16], i16, kind="ExternalInput")
    out = nc.dram_tensor("out", [n_batches, 128, D], f16, kind="ExternalOutput")

    NKT = K // 128  # 16 k-tiles
    NDT = DPAD // 128  # 5 d-tiles
    NG = NKT // 4  # 4 k-groups (one PSUM bank each)

    with tile.TileContext(nc) as tc:
        with (
            tc.tile_pool(name="const", bufs=1) as cpool,
            tc.tile_pool(name="io", bufs=2) as iopool,
            tc.tile_pool(name="gather", bufs=3) as gpool,
            tc.tile_pool(name="pt", bufs=2) as ppool,
            tc.tile_pool(name="sc_ps", bufs=3, space="PSUM") as spsum,
            tc.tile_pool(name="av_ps", bufs=2, space="PSUM") as apsum,
            tc.tile_pool(name="z_ps", bufs=2, space="PSUM") as zpsum,
            tc.tile_pool(name="outp", bufs=2) as opool,
        ):
            nc.gpsimd.load_library(mlp)
            ones = cpool.tile([128, 1], f16)
            nc.vector.memset(ones[:], 1.0)

            for b in range(n_batches):
                idx_t = iopool.tile([128, K // 16], i16, tag="idx")
                nc.sync.dma_start(idx_t[:], idx[b])
                qt_t = iopool.tile([128, DPAD], f16, tag="qt")
                nc.sync.dma_start(qt_t[:], qt[b])

                kv_rows = kvp[b * SKV : (b + 1) * SKV, :]
                # kvT[p, t, i] = kv_row(idx_i)[t*128 + p]; split across queues so
                # the per-queue Q7 pairs generate descriptors concurrently.
                KS = K // 4
                kvT = gpool.tile([128, 4, NDT, KS], f16, tag="kvT")
                for j in range(4):
                    nc.gpsimd.dma_gather(
                        kvT[:, j],
                        kv_rows,
                        idx_t[:, j * (KS // 16) : (j + 1) * (KS // 16)],
                        KS, KS, DPAD, transpose=True,
                        single_packet=False, queue_num=j,
                    )
                # v[p, j, :] = kv_row(idx_{j*128+p})[0:512]
                v = gpool.tile([128, NKT, D], f16, tag="v")
                VS = K // 2
                for j in range(2):
                    nc.gpsimd.dma_gather(
                        v[:, j * (NKT // 2) : (j + 1) * (NKT // 2), :],
                        kvp[b * SKV : (b + 1) * SKV, 0:D],
                        idx_t[:, j * (VS // 16) : (j + 1) * (VS // 16)],
                        VS, VS, D,
                        elem_step=DPAD, single_packet=False,
                        queue_num=2 * j + (b % 2),
                    )

                pT = ppool.tile([128, K], f16, tag="pT")
                for g in range(NG):
                    ps = spsum.tile([128, 512], f32, tag="ps")
                    for j in range(4):
                        i = 4 * g + j
                        for t in range(NDT):
                            nc.tensor.matmul(
                                ps[:, j * 128 : (j + 1) * 128],
                                lhsT=kvT[
                                    :, i // 4, t, (i % 4) * 128 : (i % 4 + 1) * 128
                                ],
                                rhs=qt_t[:, t * 128 : (t + 1) * 128],
                                start=(t == 0),
                                stop=(t == NDT - 1),
                            )
                    nc.scalar.activation(
                        pT[:, g * 512 : (g + 1) * 512], ps[:], Exp, scale=SM_SCALE
                    )

                av = apsum.tile([128, D], f32, tag="av")
                zz = zpsum.tile([128, 1], f32, tag="z")
                for i in range(NKT):
                    nc.tensor.matmul(
                        av[:],
                        lhsT=pT[:, i * 128 : (i + 1) * 128],
                        rhs=v[:, i, :],
                        start=(i == 0),
                        stop=(i == NKT - 1),
                    )
                    nc.tensor.matmul(
                        zz[:],
                        lhsT=pT[:, i * 128 : (i + 1) * 128],
                        rhs=ones[:],
                        start=(i == 0),
                        stop=(i == NKT - 1),
                    )

                zr = opool.tile([128, 1], f32, tag="zr")
                nc.vector.reciprocal(zr[:], zz[:])
                ob = opool.tile([128, D], f16, tag="ob")
                nc.vector.tensor_scalar_mul(ob[:], av[:], zr[:])
                nc.sync.dma_start(out[b], ob[:])
    nc.compile()
    return nc


def marshal_core_inputs(q, kv, indices, core):
    """Slice + repack one core's inputs (host-side layout marshaling only)."""
    bs = slice(core * BPC, (core + 1) * BPC)
    kvs = kv[bs, :, 0, :]  # [BPC, SKV, 576]
    kvp = np.zeros((BPC, SKV, DPAD), np.float16)
    kvp[:, :, :DK] = kvs
    kvp = kvp.reshape(BPC * SKV, DPAD)

    qs = q[bs, 0]  # [BPC, 128, 576]
    qtp = np.zeros((BPC, DPAD, 128), np.float16)
    qtp[:, :DK, :] = qs.transpose(0, 2, 1)
    # device layout: qt[b, p, t*128+h] = qT[b, t*128+p, h]
    qt = np.ascontiguousarray(
        qtp.reshape(BPC, DPAD // 128, 128, 128).transpose(0, 2, 1, 3)
    ).reshape(BPC, 128, DPAD)

    idxs = indices[bs, 0, 0, :].astype(np.int16)  # [BPC, K]
    # dma_gather layout: index i lives at partition i%16, column i//16,
    # replicated across the 8 groups of 16 partitions.
    idxw = np.ascontiguousarray(idxs.reshape(BPC, K // 16, 16).transpose(0, 2, 1))
    idxw = np.tile(idxw, (1, 8, 1))  # [BPC, 128, K//16]
    return {"kvp": kvp, "qt": qt, "idx": np.ascontiguousarray(idxw)}


def kernel(q, kv, indices):
    from concourse.bass_utils import run_bass_kernel_spmd

    q = np.asarray(q)
    kv = np.asarray(kv)
    indices = np.asarray(indices)

    if "prog" not in _PROGRAM_CACHE:
        _PROGRAM_CACHE["prog"] = build_program()
    nc = _PROGRAM_CACHE["prog"]

    in_maps = [marshal_core_inputs(q, kv, indices, c) for c in range(N_CORES)]
    res = run_bass_kernel_spmd(nc, in_maps, core_ids=list(range(N_CORES)))
    out = np.stack([r["out"] for r in res.results])  # [8, BPC, 128, 512]
    return out.reshape(B, 128, D)[:, None].astype(np.float16)
